# revision 27
# baseline (speedup 1.0000x reference)
"""GAT graph classifier on 8 Trainium2 NeuronCores.

Strategy (dst-owner sharding), v4:
  - Nodes are partitioned across 8 cores by destination ownership; each core
    owns a contiguous range of (permuted) nodes and ALL edges pointing into
    them, so per-node softmax needs no cross-core reduction.
  - Host pre-sorts edges into per-(core, block-of-128-dsts) buckets; within a
    block, edges of one dst are grouped into quads (<=4 edges sharing a dst).
    Quads are laid out full-first so trailing pad slots can be trimmed from
    the gather index streams (blocks renumbered by load rank so one static
    num_idxs works across all 8 SPMD cores).
  - Injected self-loops never enter the gather path: the self contribution
    w(v,v)*h[v] is computed locally from the resident node block.
  - Table rows carry [per-head (16 ch | 1.0)] blocks so the softmax
    denominator falls out of the same weighted gather (no V-copy).
  - The per-quad a_d values come from PE-transposed dst one-hot masks
    (transpose -> scalar-engine drain -> tiny matmul), not vector is_eq
    chains.  All PSUM reads go through the scalar (ACT) engine; the vector
    engine only touches SBUF.
  - Scatter-accumulate: one matmul per chunk (4 quad members ride the moving
    operand side by side), member groups folded by two SBUF adds.
  - Node feature tables are all-gathered between layers; graph mean-pool is a
    one-hot matmul; pooled partials are all-reduced and the tiny FC head +
    log_softmax runs redundantly on every core.
"""

import sys

sys.path.insert(0, "/opt/trn_rl_repo")

import numpy as np

import concourse.bass as bass  # noqa: F401
import concourse.bacc as bacc
import concourse.mybir as mybir
import concourse.tile as tile
from concourse import bass_utils

F32 = mybir.dt.float32
BF16 = mybir.dt.bfloat16
I16 = mybir.dt.int16
NPBF16 = mybir.dt.np(BF16)
AF = mybir.ActivationFunctionType
ALU = mybir.AluOpType

NUM_QUEUES = 4  # SWDGE queues for gathers

_AUX = {}


class Cfg:
    def __init__(self, npc, lo_cores, C_LO, C_HI, F_IN, H, C1, C2, G, NCLS):
        self.n_cores = 8
        self.npc = npc  # nodes per core (multiple of 128)
        assert npc % 128 == 0
        self.NB = npc // 128  # dst blocks per core
        self.NP = 8 * npc  # padded node count
        self.lo_cores = lo_cores
        self.LO = lo_cores * npc  # rows in the "low" gather table half
        self.HI = self.NP - self.LO
        assert self.LO < 32768 and self.HI < 32768  # int16 gather indices
        self.C_LO = C_LO  # chunks (of 128 quads) per block, low side
        self.C_HI = C_HI
        self.CC = C_LO + C_HI
        self.F_IN, self.H, self.C1 = F_IN, H, C1
        self.D1 = H * C1
        self.C2, self.G, self.NCLS = C2, G, NCLS


def full_cfg():
    return Cfg(npc=6272, lo_cores=5, C_LO=6, C_HI=4,
               F_IN=256, H=4, C1=16, C2=32, G=64, NCLS=10)


def pair_rows(cfg, nb):
    """Side-major row map for a pair of nb blocks: returns (total_rows,
    row_of(blk, c, i))."""
    NL, NHi = cfg.C_LO * 4, cfg.C_HI * 4

    def row(blk, c, i):
        if c < cfg.C_LO:
            return blk * NL + c * 4 + i
        return nb * NL + blk * NHi + (c - cfg.C_LO) * 4 + i

    return nb * (NL + NHi), row


def gad_col(cfg, nb, blk, c):
    """gad column group (side-major chunk order) for (blk, chunk)."""
    if c < cfg.C_LO:
        return blk * cfg.C_LO + c
    return nb * cfg.C_LO + blk * cfg.C_HI + (c - cfg.C_LO)


# ---------------------------------------------------------------------------
# Host-side preprocessing: sharding, quad packing, index array construction.
# ---------------------------------------------------------------------------

def _pack_blocks(cfg, ql, qh):
    npc, NB = cfg.npc, cfg.NB
    cap_lo, cap_hi = cfg.C_LO * 128, cfg.C_HI * 128
    order = np.argsort(-(ql + qh), kind="stable")
    lo_load = np.zeros(NB, np.int64)
    hi_load = np.zeros(NB, np.int64)
    nslots = np.zeros(NB, np.int64)
    block = np.empty(npc, np.int64)
    slot = np.empty(npc, np.int64)
    for d in order:
        score = np.maximum((lo_load + ql[d]) / cap_lo, (hi_load + qh[d]) / cap_hi)
        score = score + (nslots >= 128) * 1e9
        score = score + (lo_load + ql[d] > cap_lo) * 1e9
        score = score + (hi_load + qh[d] > cap_hi) * 1e9
        b = int(np.argmin(score))
        assert nslots[b] < 128 and lo_load[b] + ql[d] <= cap_lo \
            and hi_load[b] + qh[d] <= cap_hi, "packing failed; bump C_LO/C_HI"
        block[d] = b
        slot[d] = nslots[b]
        nslots[b] += 1
        lo_load[b] += ql[d]
        hi_load[b] += qh[d]
    return block, slot


def host_prep(cfg, inputs):
    x = np.asarray(inputs["x"], np.float32)
    edge_index = np.asarray(inputs["edge_index"])
    batch = np.asarray(inputs["batch"])
    N = x.shape[0]
    npc, NB, CC = cfg.npc, cfg.NB, cfg.CC
    assert N <= cfg.NP

    # NOTE: injected self-loops are handled by the on-device local path; only
    # the raw edges (including any natural self edges) go through gathers.
    src = np.asarray(edge_index[0], np.int64)
    dst = np.asarray(edge_index[1], np.int64)
    Ep = src.shape[0]

    core_d = dst // npc
    dloc = dst - core_d * npc
    side = (src // npc >= cfg.lo_cores).astype(np.int64)  # 0 lo, 1 hi

    cnt = np.zeros((8, npc, 2), np.int64)
    np.add.at(cnt, (core_d, dloc, side), 1)
    quads = -(-cnt // 4)  # ceil
    ql, qh = quads[:, :, 0], quads[:, :, 1]

    block0 = np.empty((8, npc), np.int64)
    slot = np.empty((8, npc), np.int64)
    for c in range(8):
        block0[c], slot[c] = _pack_blocks(cfg, ql[c], qh[c])

    # Renumber blocks by descending total quad load so block index b has a
    # comparable load across cores (enables a shared static num_idxs trim).
    block = np.empty_like(block0)
    for c in range(8):
        load = np.zeros(NB, np.int64)
        np.add.at(load, block0[c], ql[c] + qh[c])
        rank = np.empty(NB, np.int64)
        rank[np.argsort(-load, kind="stable")] = np.arange(NB)
        block[c] = rank[block0[c]]

    pi_local = block * 128 + slot
    inv_pi = np.empty((8, npc), np.int64)
    for c in range(8):
        inv_pi[c, pi_local[c]] = np.arange(npc)

    glob_pi = np.empty(cfg.NP, np.int64)
    ids = np.arange(cfg.NP)
    glob_pi[:] = (ids // npc) * npc + pi_local[ids // npc, ids % npc]

    # ---- quad ordering: full quads first (size desc) within (blk, side) ----
    # quad records per (core, dloc, side): sizes 4,...,4,r
    qb_g = np.zeros((8, npc, 2), np.int64)  # base quad rank of (dst, side)
    used = np.zeros((8, NB, 2), np.int64)   # slots thru last real edge
    for c in range(8):
        for s in (0, 1):
            q = quads[c, :, s]           # quads per dst
            k = cnt[c, :, s]
            bl = block[c]
            # per-quad arrays
            tot = int(q.sum())
            if tot == 0:
                continue
            d_rep = np.repeat(np.arange(npc), q)
            j_rep = np.arange(tot) - np.repeat(np.cumsum(q) - q, q)
            size = np.minimum(k[d_rep] - 4 * j_rep, 4)
            b_rep = bl[d_rep]
            # sort by (block, -size, slot, j)
            key = (b_rep * 8 + (4 - size)) * (npc * 8) + slot[c][d_rep] * 8 + j_rep
            order = np.argsort(key, kind="stable")
            # rank within block
            bo = b_rep[order]
            start = np.r_[True, bo[1:] != bo[:-1]]
            first = np.where(start)[0]
            gid = np.cumsum(start) - 1
            g = np.arange(tot) - first[gid]      # quad rank within block
            # scatter back: quad (d, j) -> g
            granks = np.empty(tot, np.int64)
            granks[order] = g
            # flat index of quad (d, j) = qb_g[d] + j
            qb_g[c, :, s] = np.cumsum(q) - q
            _GR[(c, s)] = granks
            # used slots per block: last g in block (sizes sorted desc) ->
            # slots thru last real member of last quad
            sz_o = size[order]
            # slot index of member (g, i): chunk=g//128, q=g%128 ->
            # j = (g//128)*512 + i*128 + (g%128); last member i = sz-1
            jmax = (g // 128) * 512 + (sz_o - 1) * 128 + (g % 128)
            np.maximum.at(used[c, :, s], bo, jmax + 1)

    key = (core_d * npc + dloc) * 2 + side
    order = np.argsort(key, kind="stable")
    ks = key[order]
    seg_start = np.r_[True, ks[1:] != ks[:-1]]
    seg_first = np.where(seg_start)[0]
    seg_id = np.cumsum(seg_start) - 1
    pos_in_seg = np.arange(Ep) - seg_first[seg_id]
    member = np.empty(Ep, np.int64)
    qidx = np.empty(Ep, np.int64)
    member[order] = pos_in_seg % 4
    qidx[order] = pos_in_seg // 4

    e_core = core_d
    e_blk = block[core_d, dloc]
    # quad rank g of edge: granks[(core, side)][qb_g[core, dloc, side]+qidx]
    g = np.empty(Ep, np.int64)
    for c in range(8):
        for s in (0, 1):
            m = (e_core == c) & (side == s)
            if not m.any():
                continue
            g[m] = _GR[(c, s)][qb_g[c, dloc[m], s] + qidx[m]]
    e_chunk = g // 128 + side * cfg.C_LO
    e_q = g % 128

    pi_src = glob_pi[src]
    e_val = np.where(side == 0, pi_src, pi_src - cfg.LO)
    assert (e_val >= 0).all() and (e_val < 32768).all()

    # ---- per-core gather index arrays ----
    W_LO, W_HI = 512 * cfg.C_LO, 512 * cfg.C_HI
    hlo = np.zeros((8, NB, W_LO), np.int64)
    hhi = np.zeros((8, NB, W_HI), np.int64)
    c_in_side = np.where(side == 0, e_chunk, e_chunk - cfg.C_LO)
    j = c_in_side * 512 + member * 128 + e_q
    lo_m = side == 0
    hlo[e_core[lo_m], e_blk[lo_m], j[lo_m]] = e_val[lo_m]
    hi_m = ~lo_m
    hhi[e_core[hi_m], e_blk[hi_m], j[hi_m]] = e_val[hi_m]

    # static trims: max over cores, rounded up to 16
    trim_lo = [int(min(W_LO, -(-int(used[:, b, 0].max()) // 16) * 16))
               for b in range(NB)]
    trim_hi = [int(min(W_HI, -(-int(used[:, b, 1].max()) // 16) * 16))
               for b in range(NB)]
    # never zero (ucode/ring safety)
    trim_lo = [max(t, 16) for t in trim_lo]
    trim_hi = [max(t, 16) for t in trim_hi]

    # slot mask in side-major pair row order: [q, pair-flat rows]
    NROW = CC * 4  # rows per block
    slotmask = np.zeros((8, 128, NB * NROW), np.float32)
    pair_id = e_blk // 2
    nb_of_pair = np.where(pair_id * 2 + 1 < NB, 2, 1)
    blk_in_pair = e_blk % 2
    NL, NHi = cfg.C_LO * 4, cfg.C_HI * 4
    row_lo = blk_in_pair * NL + e_chunk * 4 + member
    row_hi = nb_of_pair * NL + blk_in_pair * NHi + (e_chunk - cfg.C_LO) * 4 + member
    e_row = np.where(side == 0, row_lo, row_hi)
    pair_base = pair_id * 2 * NROW
    slotmask[e_core, e_q, pair_base + e_row] = 1.0

    # quad-level dst-slot array [q, b*CC+c]
    fm = member == 0
    dstq = np.full((8, 128, NB * CC), 200.0, np.float32)
    qc, qb2, qch, qq = e_core[fm], e_blk[fm], e_chunk[fm], e_q[fm]
    sl = slot[qc, dloc[fm]].astype(np.float32)
    dstq[qc, qq, qb2 * CC + qch] = sl

    def wrap_idx(arr):
        W = arr.shape[2]
        a = arr.reshape(8, NB, W // 16, 16).transpose(0, 3, 1, 2).reshape(8, 16, NB * W // 16)
        a = np.tile(a, (1, 8, 1)).astype(np.int16)
        return a

    hlo_w = wrap_idx(hlo)
    hhi_w = wrap_idx(hhi)

    # ---- weights ----
    W1 = np.asarray(inputs["W1"], np.float32)
    att_src1 = np.asarray(inputs["att_src1"], np.float32)
    att_dst1 = np.asarray(inputs["att_dst1"], np.float32)
    W2 = np.asarray(inputs["W2"], np.float32)
    att_src2 = np.asarray(inputs["att_src2"], np.float32)
    att_dst2 = np.asarray(inputs["att_dst2"], np.float32)
    b1 = np.asarray(inputs["b1"], np.float32)
    b2 = np.asarray(inputs["b2"], np.float32)
    fc_w = np.asarray(inputs["fc_w"], np.float32)
    fc_b = np.asarray(inputs["fc_b"], np.float32)
    H, C1, D1, C2 = cfg.H, cfg.C1, cfg.D1, cfg.C2

    As = np.zeros((D1, H), np.float32)
    Ad = np.zeros((D1, H), np.float32)
    for h in range(H):
        As[h * C1:(h + 1) * C1, h] = att_src1[h]
        Ad[h * C1:(h + 1) * C1, h] = att_dst1[h]
    W1aug = np.concatenate([W1, W1 @ As, W1 @ Ad], axis=1)  # [F_IN, D1+2H]
    W2aug = np.concatenate([W2, W2 @ att_src2[0][:, None],
                            W2 @ att_dst2[0][:, None]], axis=1)  # [D1, C2+2]

    cnt_g = np.bincount(np.asarray(batch, np.int64), minlength=cfg.G).astype(np.float32)
    invcnt = (1.0 / np.maximum(cnt_g, 1.0)).reshape(cfg.G, 1)

    KCH = -(-cfg.F_IN // 128)
    iota = np.tile(np.arange(128, dtype=np.float32), (128, 1))
    ident = np.eye(128, dtype=np.float32)
    in_maps = []
    for c in range(8):
        orig = c * npc + inv_pi[c]
        valid = orig < N
        xs = np.zeros((npc, cfg.F_IN), np.float32)
        xs[valid] = x[orig[valid]]
        xT = np.ascontiguousarray(xs.T)
        xTc = np.zeros((KCH, 128, npc), np.float32)
        for k in range(KCH):
            lo, hi = k * 128, min((k + 1) * 128, cfg.F_IN)
            xTc[k, :hi - lo] = xT[lo:hi]
        bl = np.full(npc, 255.0, np.float32)
        bl[valid] = np.asarray(batch, np.float32)[orig[valid]]
        batch_l = bl.reshape(NB, 128).T
        W1a = np.zeros((KCH, 128, D1 + 2 * H), np.float32)
        for k in range(KCH):
            lo, hi = k * 128, min((k + 1) * 128, cfg.F_IN)
            W1a[k, :hi - lo] = W1aug[lo:hi]
        # bias laid out to match the [h, 16] value view
        in_maps.append({
            "xT": xTc.astype(NPBF16),
            "W1aug": W1a.astype(NPBF16),
            "W2aug": W2aug.astype(NPBF16),
            "b1b": np.tile(b1, (128, 1)).astype(np.float32),
            "b2b": np.tile(b2, (128, 1)).astype(np.float32),
            "fcw": fc_w,
            "fcb": np.tile(fc_b, (cfg.G, 1)).astype(np.float32),
            "invcnt": invcnt,
            "iota": iota.astype(NPBF16),
            "ident": ident.astype(NPBF16),
            "hlo_idx": hlo_w[c],
            "hhi_idx": hhi_w[c],
            "dstq": dstq[c].astype(NPBF16),
            "slotmask": slotmask[c].astype(NPBF16),
            "batch_l": batch_l.astype(np.float32),
        })
    _AUX.clear()
    _AUX.update(dict(trim_lo=trim_lo, trim_hi=trim_hi))
    return in_maps, trim_lo, trim_hi


_GR = {}


# ---------------------------------------------------------------------------
# Device kernel
# ---------------------------------------------------------------------------

def build_nc(cfg, trim_lo, trim_hi):
    nc = bacc.Bacc("TRN2", target_bir_lowering=False, debug=False,
                   num_devices=cfg.n_cores, num_swdge_queues=NUM_QUEUES)
    npc, NB, CC, H, D1, C2 = cfg.npc, cfg.NB, cfg.CC, cfg.H, cfg.D1, cfg.C2
    KCH = -(-cfg.F_IN // 128)
    WAUG1 = D1 + 2 * H
    G, NCLS = cfg.G, cfg.NCLS
    C_LO, C_HI = cfg.C_LO, cfg.C_HI
    NROW = CC * 4
    NL, NHi = C_LO * 4, C_HI * 4

    xT = nc.dram_tensor("xT", [KCH, 128, npc], BF16, kind="ExternalInput")
    W1aug = nc.dram_tensor("W1aug", [KCH, 128, WAUG1], BF16, kind="ExternalInput")
    W2aug = nc.dram_tensor("W2aug", [D1, C2 + 2], BF16, kind="ExternalInput")
    b1b = nc.dram_tensor("b1b", [128, D1], F32, kind="ExternalInput")
    b2b = nc.dram_tensor("b2b", [128, C2], F32, kind="ExternalInput")
    fcw = nc.dram_tensor("fcw", [C2, NCLS], F32, kind="ExternalInput")
    fcb = nc.dram_tensor("fcb", [G, NCLS], F32, kind="ExternalInput")
    invcnt = nc.dram_tensor("invcnt", [G, 1], F32, kind="ExternalInput")
    iota_d = nc.dram_tensor("iota", [128, 128], BF16, kind="ExternalInput")
    ident_d = nc.dram_tensor("ident", [128, 128], BF16, kind="ExternalInput")
    WL, WH = 512 * C_LO // 16, 512 * C_HI // 16
    hlo_d = nc.dram_tensor("hlo_idx", [128, NB * WL], I16, kind="ExternalInput")
    hhi_d = nc.dram_tensor("hhi_idx", [128, NB * WH], I16, kind="ExternalInput")
    dstq_d = nc.dram_tensor("dstq", [128, NB * CC], BF16, kind="ExternalInput")
    slotm_d = nc.dram_tensor("slotmask", [128, NB * NROW], BF16, kind="ExternalInput")
    batch_d = nc.dram_tensor("batch_l", [128, NB], F32, kind="ExternalInput")
    out_d = nc.dram_tensor("out", [G, NCLS], F32, kind="ExternalOutput")

    qload = [0] * NUM_QUEUES

    def pick_queue(n):
        q = min(range(NUM_QUEUES), key=lambda i: qload[i])
        qload[q] += n
        return q

    with tile.TileContext(nc) as tc:
        with tc.tile_pool(name="dram", bufs=1, space="DRAM") as dram, \
             tc.tile_pool(name="const", bufs=1) as const:
            h1own = dram.tile([npc, 128], BF16)
            h2own = dram.tile([npc, 128], BF16)
            h1full = dram.tile([cfg.NP, 128], BF16, addr_space="Shared")
            h2full = dram.tile([cfg.NP, 128], BF16, addr_space="Shared")
            poolin = dram.tile([C2, G], F32)
            poolout = dram.tile([C2, G], F32, addr_space="Shared")

            iota_sb = const.tile([128, 128], BF16)
            ident_sb = const.tile([128, 128], BF16)
            zeros_sb = const.tile([128, 128], F32)
            nc.vector.memset(zeros_sb[:], 0)
            dstq_sb = const.tile([128, NB * CC], BF16)
            slotm_sb = const.tile([128, NB * NROW], BF16)
            batch_sb = const.tile([128, NB], F32)
            b1b_sb = const.tile([128, D1], F32)
            b2b_sb = const.tile([128, C2], F32)
            invc_sb = const.tile([G, 1], F32)
            fcw_sb = const.tile([C2, NCLS], F32)
            fcb_sb = const.tile([G, NCLS], F32)
            W2aug_sb = const.tile([D1, C2 + 2], BF16)
            hlo_sb = const.tile([128, NB * WL], I16)
            hhi_sb = const.tile([128, NB * WH], I16)
            # persistent per-node staging tables (row layout: see edge_layer)
            stage1 = const.tile([128, NB * 128], BF16)
            stage2 = const.tile([128, NB * 128], BF16)
            hl1_sb = const.tile([128, NB * D1], BF16)
            hout_sb = const.tile([128, NB * C2], BF16)
            ws1 = const.tile([128, NB * H], F32)
            ws2 = const.tile([128, NB], F32)
            pin_sb = const.tile([C2, G], F32)

            for sb, d in [(iota_sb, iota_d), (ident_sb, ident_d),
                          (dstq_sb, dstq_d), (slotm_sb, slotm_d),
                          (batch_sb, batch_d), (b1b_sb, b1b), (b2b_sb, b2b),
                          (invc_sb, invcnt), (fcw_sb, fcw), (fcb_sb, fcb),
                          (W2aug_sb, W2aug), (hlo_sb, hlo_d), (hhi_sb, hhi_d)]:
                nc.sync.dma_start(sb[:], d[:])

            # ---------------- phase A: stage1 = x @ W1aug ----------------
            # stage1 row: [(h 16 | 1.0) x4 | a_s x4 | a_d x4 | junk]
            with tc.tile_pool(name="phA", bufs=1) as phA, \
                 tc.tile_pool(name="psA", bufs=4, space="PSUM") as psA:
                xT_sb = phA.tile([128, KCH * npc], BF16)
                W1a_sb = phA.tile([128, KCH * WAUG1], BF16)
                for k in range(KCH):
                    nc.sync.dma_start(xT_sb[:, k * npc:(k + 1) * npc], xT[k])
                    nc.sync.dma_start(W1a_sb[:, k * WAUG1:(k + 1) * WAUG1], W1aug[k])
                for t in range(NB):
                    ps = psA.tile([128, WAUG1], F32, tag="psa")
                    for k in range(KCH):
                        nc.tensor.matmul(
                            ps[:],
                            xT_sb[:, k * npc + t * 128: k * npc + (t + 1) * 128],
                            W1a_sb[:, k * WAUG1:(k + 1) * WAUG1],
                            start=(k == 0), stop=(k == KCH - 1))
                    nc.scalar.copy(
                        stage1[:, t * 128: t * 128 + 68]
                        .rearrange("p (h y) -> p h y", y=17)[:, :, 0:16],
                        ps[:, 0:D1].rearrange("p (h y) -> p h y", y=16))
                    nc.scalar.copy(
                        stage1[:, t * 128 + 68: t * 128 + 76],
                        ps[:, D1:D1 + 2 * H])
                s2v = stage2[:].rearrange("p (t c) -> p t c", c=128)
                nc.vector.memset(s2v[:, :, 32:33], 1.0)
                s1v = stage1[:].rearrange("p (t c) -> p t c", c=128)
                nc.vector.memset(
                    s1v[:, :, 0:68].rearrange("p t (h y) -> p t h y", y=17)
                    [:, :, :, 16:17], 1.0)
                nc.sync.dma_start(
                    h1own[:].rearrange("(t p) c -> p t c", p=128),
                    stage1[:].rearrange("p (t c) -> p t c", c=128))
            # persistent gather-destination pool (zeroed once; trimmed tail
            # slots keep stale-but-finite values which wb zeroes out)
            gp_cm = tc.tile_pool(name="gp", bufs=5)
            gp = gp_cm.__enter__()
            GB = 5
            for _ in range(GB):
                glo = gp.tile([128, 2 * NL * 128], BF16, tag="glo")
                ghi = gp.tile([128, 2 * NHi * 128], BF16, tag="ghi")
                nc.vector.memset(glo[:], 0)
                nc.vector.memset(ghi[:], 0)

            nc.gpsimd.collective_compute(
                "AllGather", ALU.bypass,
                replica_groups=[list(range(cfg.n_cores))],
                ins=[h1own[:].opt()], outs=[h1full[:].opt()])

            def edge_layer(layer):
                if layer == 1:
                    htab, NH, CW = h1full, H, 17
                    ACOL, stage, bias_sb, out_sb, ws = 68, stage1, b1b_sb, hl1_sb, ws1
                    DV = D1
                else:
                    htab, NH, CW = h2full, 1, 33
                    ACOL, stage, bias_sb, out_sb, ws = 33, stage2, b2b_sb, hout_sb, ws2
                    DV = C2
                D = NH * CW          # V row width (values + inline denom)
                ADCOL = ACOL + NH
                VC = CW - 1          # value cols per head

                # self-loop weights for every owned node (batched)
                nc.vector.tensor_tensor(
                    ws[:].rearrange("p (t h) -> p t h", h=NH),
                    stage[:].rearrange("p (t c) -> p t c", c=128)
                    [:, :, ACOL:ACOL + NH],
                    stage[:].rearrange("p (t c) -> p t c", c=128)
                    [:, :, ADCOL:ADCOL + NH],
                    ALU.add)
                nc.vector.scalar_tensor_tensor(
                    ws[:], ws[:], 0.2, ws[:], ALU.mult, ALU.max)
                nc.scalar.activation(ws[:], ws[:], AF.Exp)

                with tc.tile_pool(name=f"ve{layer}", bufs=2) as vp, \
                     tc.tile_pool(name=f"pst{layer}", bufs=2, space="PSUM") as psT, \
                     tc.tile_pool(name=f"psg{layer}", bufs=1, space="PSUM") as psg, \
                     tc.tile_pool(name=f"pse{layer}", bufs=2, space="PSUM") as pse, \
                     tc.tile_pool(name=f"psp{layer}", bufs=1, space="PSUM") as psp:
                    psum_pool = None
                    if layer == 2:
                        psum_pool = psp.tile([C2, G], F32, tag="pool")
                    for b0 in range(0, NB, 2):
                        nb = 2 if b0 + 1 < NB else 1
                        NRP, row = pair_rows(cfg, nb)
                        glo = gp.tile([128, 2 * NL * 128], BF16, tag="glo")
                        ghi = gp.tile([128, 2 * NHi * 128], BF16, tag="ghi")

                        def emit_gather(dview, tab, idx, base_col, trim):
                            n1 = min(trim, -(-(trim // 2) // 128) * 128)
                            for off, cnt in ((0, n1), (n1, trim - n1)):
                                if cnt <= 0:
                                    continue
                                nt = -(-cnt // 128)
                                nc.gpsimd.dma_gather(
                                    dview[:, off // 128: off // 128 + nt],
                                    tab,
                                    idx[:, base_col + off // 16:],
                                    num_idxs=cnt, num_idxs_reg=cnt,
                                    elem_size=128, single_packet=False,
                                    queue_num=pick_queue(cnt))

                        for k in range(nb):
                            b = b0 + k
                            emit_gather(
                                glo[:, k * NL * 128:(k + 1) * NL * 128]
                                .rearrange("p (n e) -> p n e", e=128),
                                htab[0:cfg.LO, :], hlo_sb, b * WL, trim_lo[b])
                            emit_gather(
                                ghi[:, k * NHi * 128:(k + 1) * NHi * 128]
                                .rearrange("p (n e) -> p n e", e=128),
                                htab[cfg.LO:cfg.NP, :], hhi_sb, b * WH,
                                trim_hi[b])

                        # dst one-hot masks for the pair's chunks in one op
                        mask = vp.tile([128, 2 * CC * 128], BF16, tag="mask")
                        nc.vector.tensor_tensor(
                            mask[:, 0:nb * CC * 128]
                            .rearrange("p (c e) -> p c e", e=128),
                            dstq_sb[:, b0 * CC:(b0 + nb) * CC]
                            .unsqueeze(2).broadcast_to((128, nb * CC, 128)),
                            iota_sb[:].unsqueeze(1)
                            .broadcast_to((128, nb * CC, 128)),
                            ALU.is_equal)

                        # per-quad a_d: PE-transpose each chunk mask (4 chunks
                        # share a PSUM bank), drain via ACT, then one tiny
                        # matmul per chunk
                        gad = psg.tile([128, 2 * CC * NH], F32, tag="gad")
                        chunks = [(k, c) for k in range(nb) for c in range(CC)]
                        for j0 in range(0, len(chunks), 4):
                            grp = chunks[j0:j0 + 4]
                            pt = psT.tile([128, 4 * 128], BF16, tag="pt")
                            for gi, (k, c) in enumerate(grp):
                                nc.tensor.transpose(
                                    pt[:, gi * 128:(gi + 1) * 128],
                                    mask[:, (k * CC + c) * 128:
                                         (k * CC + c + 1) * 128],
                                    ident_sb[:])
                            mtd = vp.tile([128, 4 * 128], BF16, tag="mtd",
                                          bufs=2)
                            nc.scalar.copy(mtd[:, 0:len(grp) * 128],
                                           pt[:, 0:len(grp) * 128])
                            for gi, (k, c) in enumerate(grp):
                                gc = gad_col(cfg, nb, k, c)
                                nc.tensor.matmul(
                                    gad[:, gc * NH:(gc + 1) * NH],
                                    mtd[:, gi * 128:(gi + 1) * 128],
                                    stage[:, (b0 + k) * 128 + ADCOL:
                                          (b0 + k) * 128 + ADCOL + NH],
                                    start=True, stop=True)
                        gadd = vp.tile([128, 2 * CC * NH], F32, tag="gadd",
                                       bufs=1)
                        nc.scalar.copy(gadd[:, 0:nb * CC * NH],
                                       gad[:, 0:nb * CC * NH])

                        # scores z = a_s[src] + a_d[dst]  (side-major rows)
                        z = vp.tile([128, NRP * NH], F32, tag="z")
                        gadv = gadd[:].rearrange("p (g h) -> p g h", h=NH)
                        for gt, r0, ng in ((glo, 0, nb * C_LO),
                                           (ghi, nb * C_LO, nb * C_HI)):
                            nc.vector.tensor_tensor(
                                z[:].rearrange("p (g i h) -> p g i h", i=4, h=NH)
                                [:, r0:r0 + ng],
                                gt[:].rearrange("p (g i e) -> p g i e",
                                                i=4, e=128)
                                [:, 0:ng, :, ACOL:ACOL + NH],
                                gadv[:, r0:r0 + ng]
                                .unsqueeze(2).broadcast_to((128, ng, 4, NH)),
                                ALU.add)
                        # w = exp(leaky_relu(z, 0.2)); exact lrelu on vector,
                        # exp on the ACT engine (its table stays resident)
                        nc.vector.scalar_tensor_tensor(
                            z[:], z[:], 0.2, z[:], ALU.mult, ALU.max)
                        nc.scalar.activation(z[:], z[:], AF.Exp)
                        w = z
                        wb = vp.tile([128, NRP * NH], BF16, tag="wb")
                        sm = slotm_sb[:, b0 * NROW: b0 * NROW + NRP]
                        if NH > 1:
                            nc.vector.tensor_tensor(
                                wb[:].rearrange("p (n h) -> p n h", h=NH),
                                w[:].rearrange("p (n h) -> p n h", h=NH),
                                sm.unsqueeze(2).broadcast_to((128, NRP, NH)),
                                ALU.mult)
                        else:
                            nc.vector.tensor_tensor(wb[:], w[:], sm, ALU.mult)

                        # V = wb * rows (per-head 17-col blocks incl ones col)
                        V = vp.tile([128, NRP * D], BF16, tag="V", bufs=1)
                        wb3 = wb[:].rearrange("p (n h) -> p n h", h=NH)
                        for gt, n0, nn in ((glo, 0, nb * NL),
                                           (ghi, nb * NL, nb * NHi)):
                            nc.vector.tensor_tensor(
                                V[:].rearrange("p (n w) -> p n w", w=D)
                                [:, n0:n0 + nn]
                                .rearrange("p n (h y) -> p n h y", y=CW),
                                gt[:].rearrange("p (n e) -> p n e", e=128)
                                [:, 0:nn, 0:D]
                                .rearrange("p n (h y) -> p n h y", y=CW),
                                wb3[:, n0:n0 + nn]
                                .unsqueeze(3).broadcast_to((128, nn, NH, CW)),
                                ALU.mult)

                        # scatter-accumulate: one matmul per chunk, 4 members
                        # side by side on the moving operand
                        o_pair = vp.tile([128, 2 * D], F32, tag="opair")
                        for k in range(nb):
                            ps = pse.tile([128, 4 * D], F32, tag="pse")
                            for c in range(CC):
                                r0 = row(k, c, 0)
                                nc.tensor.matmul(
                                    ps[:],
                                    mask[:, (k * CC + c) * 128:
                                         (k * CC + c + 1) * 128],
                                    V[:, r0 * D:(r0 + 4) * D],
                                    start=(c == 0), stop=(c == CC - 1))
                            o4 = vp.tile([128, 4 * D], F32, tag="o4", bufs=2)
                            nc.scalar.copy(o4[:], ps[:])
                            nc.vector.tensor_tensor(
                                o4[:, 0:2 * D], o4[:, 0:2 * D],
                                o4[:, 2 * D:4 * D], ALU.add)
                            nc.vector.tensor_tensor(
                                o_pair[:, k * D:(k + 1) * D],
                                o4[:, 0:D], o4[:, D:2 * D], ALU.add)

                        # self-loop contribution + epilogue (batched per pair)
                        sv = stage[:].rearrange("p (t c) -> p t c", c=128)
                        tmp = vp.tile([128, 2 * D], F32, tag="tmp")
                        nc.vector.tensor_tensor(
                            tmp[:, 0:nb * D]
                            .rearrange("p (n h y) -> p n h y", h=NH, y=CW),
                            sv[:, b0:b0 + nb, 0:D]
                            .rearrange("p n (h y) -> p n h y", y=CW),
                            ws[:].rearrange("p (t h) -> p t h", h=NH)
                            [:, b0:b0 + nb]
                            .unsqueeze(3).broadcast_to((128, nb, NH, CW)),
                            ALU.mult)
                        nc.vector.tensor_tensor(
                            o_pair[:, 0:nb * D], o_pair[:, 0:nb * D],
                            tmp[:, 0:nb * D], ALU.add)
                        opv = o_pair[:].rearrange("p (n h y) -> p n h y",
                                                  h=NH, y=CW)
                        rec = vp.tile([128, 2 * NH], F32, tag="rec")
                        nc.vector.reciprocal(
                            rec[:].rearrange("p (n h) -> p n h", h=NH)
                            [:, 0:nb].unsqueeze(3),
                            opv[:, 0:nb, :, VC:CW])
                        oh = vp.tile([128, 2 * DV], F32, tag="oh")
                        ohv = oh[:].rearrange("p (n h y) -> p n h y",
                                              h=NH, y=VC)
                        nc.vector.tensor_tensor(
                            ohv[:, 0:nb],
                            opv[:, 0:nb, :, 0:VC],
                            rec[:].rearrange("p (n h) -> p n h", h=NH)
                            [:, 0:nb].unsqueeze(3)
                            .broadcast_to((128, nb, NH, VC)),
                            ALU.mult)
                        nc.vector.tensor_tensor(
                            ohv[:, 0:nb], ohv[:, 0:nb],
                            bias_sb[:].rearrange("p (h y) -> p h y", y=VC)
                            .unsqueeze(1).broadcast_to((128, nb, NH, VC)),
                            ALU.add)
                        m = vp.tile([128, 2 * DV], F32, tag="m", bufs=1)
                        nc.vector.tensor_tensor(
                            m[:, 0:nb * DV], oh[:, 0:nb * DV],
                            zeros_sb[:, 0:nb * DV], ALU.min)
                        nc.scalar.activation(m[:, 0:nb * DV], m[:, 0:nb * DV],
                                             AF.Exp)
                        nc.vector.scalar_tensor_tensor(
                            out_sb[:, b0 * DV:(b0 + nb) * DV],
                            m[:, 0:nb * DV], -1.0, oh[:, 0:nb * DV],
                            ALU.add, ALU.max)

                        if layer == 2:
                            # fold graph mean-pool accumulation into the loop
                            for k in range(nb):
                                b = b0 + k
                                mp = vp.tile([128, G], BF16, tag="mp")
                                nc.vector.tensor_tensor(
                                    mp[:], iota_sb[:, 0:G],
                                    batch_sb[:, b:b + 1]
                                    .broadcast_to((128, G)), ALU.is_equal)
                                nc.tensor.matmul(
                                    psum_pool[:],
                                    hout_sb[:, b * C2:(b + 1) * C2], mp[:],
                                    start=(b == 0), stop=(b == NB - 1))

                        if layer == 1:
                            # fold the layer-2 projection into the loop:
                            # stage2 row: [h2 32 | 1.0 | a_s | a_d | junk]
                            for k in range(nb):
                                b = b0 + k
                                pt2 = psT.tile([D1, 128], BF16, tag="pt2",
                                               bufs=1)
                                nc.tensor.transpose(
                                    pt2[:], hl1_sb[:, b * D1:(b + 1) * D1],
                                    ident_sb[:])
                                t2 = vp.tile([D1, 128], BF16, tag="t2", bufs=2)
                                nc.scalar.copy(t2[:], pt2[:])
                                p2 = psp.tile([128, C2 + 2], F32, tag="p2")
                                nc.tensor.matmul(p2[:], t2[:], W2aug_sb[:],
                                                 start=True, stop=True)
                                nc.scalar.copy(
                                    stage2[:, b * 128: b * 128 + C2],
                                    p2[:, 0:C2])
                                nc.scalar.copy(
                                    stage2[:, b * 128 + 33: b * 128 + 35],
                                    p2[:, C2:C2 + 2])

                    if layer == 1:
                        nc.sync.dma_start(
                            h2own[:].rearrange("(t p) c -> p t c", p=128),
                            stage2[:].rearrange("p (t c) -> p t c", c=128))
                    if layer == 2:
                        nc.scalar.copy(pin_sb[:], psum_pool[:])

            edge_layer(1)
            nc.gpsimd.collective_compute(
                "AllGather", ALU.bypass,
                replica_groups=[list(range(cfg.n_cores))],
                ins=[h2own[:].opt()], outs=[h2full[:].opt()])
            edge_layer(2)

            # ---------------- pooling + head ----------------
            with tc.tile_pool(name="pool", bufs=2) as pp, \
                 tc.tile_pool(name="psL", bufs=1, space="PSUM") as psL:
                nc.sync.dma_start(poolin[:], pin_sb[:])
                nc.gpsimd.collective_compute(
                    "AllReduce", ALU.add,
                    replica_groups=[list(range(cfg.n_cores))],
                    ins=[poolin[:].opt()], outs=[poolout[:].opt()])
                pout_sb = pp.tile([C2, G], F32)
                nc.sync.dma_start(pout_sb[:], poolout[:])
                psl = psL.tile([G, NCLS], F32)
                nc.tensor.matmul(psl[:], pout_sb[:], fcw_sb[:],
                                 start=True, stop=True)
                L = pp.tile([G, NCLS], F32)
                nc.scalar.copy(L[:], psl[:])
                nc.vector.tensor_scalar(L[:], L[:], invc_sb[:], None, ALU.mult)
                nc.vector.tensor_tensor(L[:], L[:], fcb_sb[:], ALU.add)
                mx = pp.tile([G, 1], F32)
                nc.vector.tensor_reduce(mx[:], L[:], mybir.AxisListType.X, ALU.max)
                nc.vector.tensor_scalar(L[:], L[:], mx[:], None, ALU.subtract)
                ex = pp.tile([G, NCLS], F32)
                se = pp.tile([G, 1], F32)
                nc.scalar.activation(ex[:], L[:], AF.Exp, accum_out=se[:])
                lse = pp.tile([G, 1], F32)
                nc.scalar.activation(lse[:], se[:], AF.Ln)
                outL = pp.tile([G, NCLS], F32)
                nc.vector.tensor_scalar(outL[:], L[:], lse[:], None, ALU.subtract)
                nc.sync.dma_start(out_d[:], outL[:])
            gp_cm.__exit__(None, None, None)
    nc.compile()
    return nc


# ---------------------------------------------------------------------------
# Entry point
# ---------------------------------------------------------------------------

_NC_CACHE = {}


def kernel(**inputs):
    cfg = full_cfg()
    in_maps, trim_lo, trim_hi = host_prep(cfg, inputs)
    key = (tuple(trim_lo), tuple(trim_hi))
    if _NC_CACHE.get("key") != key:
        _NC_CACHE["nc"] = build_nc(cfg, trim_lo, trim_hi)
        _NC_CACHE["key"] = key
    nc = _NC_CACHE["nc"]
    res = bass_utils.run_bass_kernel_spmd(
        nc, in_maps, core_ids=list(range(cfg.n_cores)))
    return np.asarray(res.results[0]["out"], np.float32)
